# revision 1
# baseline (speedup 1.0000x reference)
"""Trainium2 Bass kernel for nn_Conv2d_71476845740806.

Reference semantics (buggy naive Conv2d):
  xsum = pad(input, 1).sum(batch)                  # (1, C, 258, 258)
  conv = conv2d(xsum, weight, stride=2, VALID)     # (1, K, 128, 128)
  vals = conv[0, :, :64, :64]                      # (K, 64, 64)
  out  = zeros(B, K, 128, 128); out[:, :, ::2, ::2] = vals  (batch-replicated)

Only window starts (2i, 2j), i,j in [0,64) are used -> only padded rows/cols
0..128 of the summed image matter -> only input rows/cols 0..127.

Device strategy (8 cores, SPMD):
  - Shard the 64 output rows: core q computes rows 8q..8q+7 for ALL K=128
    filters. Needs padded rows 16q..16q+16 (17 rows) x 129 cols, all b, c.
  - Host preps per-core tensors: xe (even padded rows: 8,64,9,130) and
    xo (odd padded rows: 8,64,8,130), zero-padded already.
  - SBUF layout: partitions 0..63 = (c, even rows), 64..127 = (c, odd rows).
    Batch-sum via DVE tensor_add tree (7 adds, fp32).
  - Conv as 6 matmuls accumulating in one PSUM bank [128, 512]:
      3 x contract-128 (kernel rows dh=0,1 paired across partition halves)
      3 x contract-64  (dh=2, even partitions only, shifted one row)
    rhs AP does the stride-2 column access directly: [.., (8 rows), (64 cols step 2)].
  - DMA out per-core vals (128, 8*64); host scatters into the zero output.
"""

import numpy as np

import concourse.bacc as bacc
import concourse.bass as bass
import concourse.mybir as mybir
from concourse import tile
from concourse.bass_utils import run_bass_kernel_spmd

F32 = mybir.dt.float32

B, C, H, W = 8, 64, 256, 256
K = 128
NCORES = 8
ROWS_PER_CORE = 8          # output rows per core (64 total)
ER = 9                     # even padded rows per core
OR = 8                     # odd padded rows per core
WCOLS = 130                # stored padded cols 0..129 (used: 0..128)
PITCH = ER * WCOLS         # 1170 per-batch free pitch
NOUT = ROWS_PER_CORE * 64  # 512

# Matmul input dtype: "fp32" (exact, 4 cyc/row) or "f32r" (1 cyc/row).
MM_DTYPE = "fp32"
# Strategy: "tree" = DVE sum tree then 6 matmuls on xsum;
#           "pair" = 4 DVE pair-adds, 6 matmuls per pair (24 total);
#           "batch" = no DVE, 6 matmuls per batch (48 total).
STRATEGY = "tree"

TRACE = False
LAST_EXEC_NS = None

_cache = {}


def _mm_cast(ap):
    if MM_DTYPE == "f32r":
        return ap.bitcast(mybir.dt.float32r)
    return ap


def _emit_conv_matmuls(nc, wpt3, w2t3, src3, psum3, start, stop):
    """6 matmuls of the 3x3 stride-2 conv of src into psum (accumulating).

    src3: [128, ER, WCOLS] AP; partitions 0..63 = (c, even rows),
          64..127 = (c, odd rows; row index i is padded row 2i+1).
    """
    for dw in range(3):
        # dh=0 (even rows, row i) paired with dh=1 (odd rows, row i)
        nc.tensor.matmul(
            psum3[:, :, :],
            _mm_cast(wpt3[:, dw, :]),
            _mm_cast(src3[:, 0:ROWS_PER_CORE, dw : dw + 128 : 2]),
            start=(start and dw == 0),
            stop=False,
        )
    for dw in range(3):
        # dh=2: even rows, row i+1
        nc.tensor.matmul(
            psum3[0:K, :, :],
            _mm_cast(w2t3[0:64, dw, :]),
            _mm_cast(src3[0:64, 1 : 1 + ROWS_PER_CORE, dw : dw + 128 : 2]),
            start=False,
            stop=(stop and dw == 2),
        )


def _build_program():
    key = (MM_DTYPE, STRATEGY)
    if key in _cache:
        return _cache[key]

    nc = bacc.Bacc(None)
    xe = nc.declare_dram_parameter("xe", [B, C, ER, WCOLS], F32, isOutput=False)
    xo = nc.declare_dram_parameter("xo", [B, C, OR, WCOLS], F32, isOutput=False)
    wp = nc.declare_dram_parameter("wp", [128, 3, K], F32, isOutput=False)
    w2 = nc.declare_dram_parameter("w2", [64, 3, K], F32, isOutput=False)
    out = nc.declare_dram_parameter("out", [K, NOUT], F32, isOutput=True)

    with tile.TileContext(nc) as tc:
        with (
            tc.tile_pool(name="sbuf", bufs=1) as pool,
            tc.tile_pool(name="psum", bufs=1, space="PSUM") as ppool,
        ):
            wpt = pool.tile([128, 3 * K], F32, tag="wpt")
            w2t = pool.tile([64, 3 * K], F32, tag="w2t")
            nc.sync.dma_start(out=wpt[:, :], in_=wp[:, :, :].rearrange("p a b -> p (a b)"))
            nc.sync.dma_start(out=w2t[:, :], in_=w2[:, :, :].rearrange("p a b -> p (a b)"))
            wpt3 = wpt[:, :].rearrange("p (a b) -> p a b", a=3)
            w2t3 = w2t[:, :].rearrange("p (a b) -> p a b", a=3)

            staging = pool.tile([128, B * PITCH], F32, tag="staging")
            st3 = staging[:, :].rearrange("p (b x) -> p b x", b=B)
            xer = xe[:, :, :, :].rearrange("b c r w -> c b (r w)")
            xor_ = xo[:, :, :, :].rearrange("b c r w -> c b (r w)")
            for bp in range(4):
                b0 = 2 * bp
                nc.sync.dma_start(
                    out=st3[0:64, b0 : b0 + 2, :], in_=xer[:, b0 : b0 + 2, :]
                )
                nc.sync.dma_start(
                    out=st3[64:128, b0 : b0 + 2, 0 : OR * WCOLS],
                    in_=xor_[:, b0 : b0 + 2, :],
                )

            psum = ppool.tile([K, NOUT], F32, tag="psum")
            psum3 = psum[:, :].rearrange("p (r w) -> p r w", r=ROWS_PER_CORE)

            if STRATEGY == "batch":
                for b in range(B):
                    src3 = st3[:, b, :].rearrange("p (r w) -> p r w", r=ER)
                    _emit_conv_matmuls(
                        nc, wpt3, w2t3, src3, psum3, start=(b == 0), stop=(b == B - 1)
                    )
            else:
                spair = pool.tile([128, 4 * PITCH], F32, tag="spair")
                sp3 = spair[:, :].rearrange("p (b x) -> p b x", b=4)
                for bp in range(4):
                    nc.vector.tensor_add(
                        sp3[:, bp, :], st3[:, 2 * bp, :], st3[:, 2 * bp + 1, :]
                    )
                if STRATEGY == "pair":
                    for bp in range(4):
                        src3 = sp3[:, bp, :].rearrange("p (r w) -> p r w", r=ER)
                        _emit_conv_matmuls(
                            nc, wpt3, w2t3, src3, psum3,
                            start=(bp == 0), stop=(bp == 3),
                        )
                else:  # tree
                    t01 = pool.tile([128, PITCH], F32, tag="t01")
                    t23 = pool.tile([128, PITCH], F32, tag="t23")
                    xsum = pool.tile([128, PITCH], F32, tag="xsum")
                    nc.vector.tensor_add(t01[:, :], sp3[:, 0, :], sp3[:, 1, :])
                    nc.vector.tensor_add(t23[:, :], sp3[:, 2, :], sp3[:, 3, :])
                    nc.vector.tensor_add(xsum[:, :], t01[:, :], t23[:, :])
                    src3 = xsum[:, :].rearrange("p (r w) -> p r w", r=ER)
                    _emit_conv_matmuls(
                        nc, wpt3, w2t3, src3, psum3, start=True, stop=True
                    )

            outs = pool.tile([K, NOUT], F32, tag="outs")
            nc.scalar.copy(outs[:, :], psum[:, :])
            nc.sync.dma_start(out=out[:, :], in_=outs[:, :])

    nc.compile()
    _cache[key] = nc
    return nc


def _prep_inputs(input, weight):
    inp = np.ascontiguousarray(input, dtype=np.float32)
    w = np.ascontiguousarray(weight, dtype=np.float32)

    # Padded top-left region: P[r, w] = padded coord (orig r-1, w-1)
    P = np.zeros((B, C, 130, WCOLS), np.float32)
    P[:, :, 1:129, 1:129] = inp[:, :, :128, :128]

    t = [np.ascontiguousarray(w[:, :, dh, :].transpose(1, 2, 0)) for dh in range(3)]
    wp_host = np.ascontiguousarray(np.concatenate([t[0], t[1]], axis=0))
    w2_host = t[2]

    in_maps = []
    for q in range(NCORES):
        r0 = 16 * q
        xe_q = np.ascontiguousarray(P[:, :, r0 : r0 + 17 : 2, :])
        xo_q = np.ascontiguousarray(P[:, :, r0 + 1 : r0 + 16 : 2, :])
        in_maps.append({"xe": xe_q, "xo": xo_q, "wp": wp_host, "w2": w2_host})
    return in_maps


def kernel(input, weight):
    global LAST_EXEC_NS
    nc = _build_program()
    in_maps = _prep_inputs(input, weight)
    res = run_bass_kernel_spmd(nc, in_maps, list(range(NCORES)), trace=TRACE)
    LAST_EXEC_NS = res.exec_time_ns

    vals = np.concatenate(
        [res.results[q]["out"].reshape(K, ROWS_PER_CORE, 64) for q in range(NCORES)],
        axis=1,
    )  # (K, 64, 64)
    out = np.zeros((B, K, 128, 128), np.float32)
    out[:, :, ::2, ::2] = vals[None]
    return out


# revision 2
# speedup vs baseline: 1.0236x; 1.0236x over previous
"""Trainium2 Bass kernel for nn_Conv2d_71476845740806.

Reference semantics (buggy naive Conv2d):
  xsum = pad(input, 1).sum(batch)                  # (1, C, 258, 258)
  conv = conv2d(xsum, weight, stride=2, VALID)     # (1, K, 128, 128)
  vals = conv[0, :, :64, :64]                      # (K, 64, 64)
  out  = zeros(B, K, 128, 128); out[:, :, ::2, ::2] = vals  (batch-replicated)

Only window starts (2i, 2j), i,j in [0,64) are used -> only padded rows/cols
0..128 of the summed image matter -> only input rows/cols 0..127.

Device strategy (8 cores, SPMD):
  - Shard the 64 output rows: core q computes rows 8q..8q+7 for ALL K=128
    filters. Needs padded rows 16q..16q+16 (17 rows) x 129 cols, all b, c.
  - Host preps per-core tensors: xe (even padded rows: 8,64,9,130) and
    xo (odd padded rows: 8,64,8,130), zero-padded already.
  - SBUF layout: partitions 0..63 = (c, even rows), 64..127 = (c, odd rows).
    Batch-sum via DVE tensor_add tree (7 adds, fp32).
  - Conv as 6 matmuls accumulating in one PSUM bank [128, 512]:
      3 x contract-128 (kernel rows dh=0,1 paired across partition halves)
      3 x contract-64  (dh=2, even partitions only, shifted one row)
    rhs AP does the stride-2 column access directly: [.., (8 rows), (64 cols step 2)].
  - DMA out per-core vals (128, 8*64); host scatters into the zero output.
"""

import numpy as np

import concourse.bacc as bacc
import concourse.bass as bass
import concourse.mybir as mybir
from concourse import tile
from concourse.bass_utils import run_bass_kernel_spmd

F32 = mybir.dt.float32
F32R = mybir.dt.float32r

B, C, H, W = 8, 64, 256, 256
K = 128
NCORES = 8
ROWS_PER_CORE = 8          # output rows per core (64 total)
ER = 9                     # even padded rows per core
OR = 8                     # odd padded rows per core
WCOLS = 130                # stored padded cols 0..129 (used: 0..128)
PITCH = ER * WCOLS         # 1170 per-batch free pitch
NOUT = ROWS_PER_CORE * 64  # 512

# Matmul input dtype: "fp32" (exact, 4 cyc/row) or "f32r" (1 cyc/row).
MM_DTYPE = "fp32"
# Strategy: "tree" = DVE sum tree then 6 matmuls on xsum;
#           "pair" = 4 DVE pair-adds, 6 matmuls per pair (24 total);
#           "batch" = no DVE, 6 matmuls per batch (48 total).
STRATEGY = "tree"

TRACE = False
LAST_EXEC_NS = None

_cache = {}


def _emit_conv_matmuls(nc, wpt3, w2t3, src3, psum3, start, stop):
    """6 matmuls of the 3x3 stride-2 conv of src into psum (accumulating).

    src3: [128, ER, WCOLS] AP; partitions 0..63 = (c, even rows),
          64..127 = (c, odd rows; row index i is padded row 2i+1).
    """
    for dw in range(3):
        # dh=0 (even rows, row i) paired with dh=1 (odd rows, row i)
        nc.tensor.matmul(
            psum3[:, :, :],
            wpt3[:, dw, :],
            src3[:, 0:ROWS_PER_CORE, dw : dw + 128 : 2],
            start=(start and dw == 0),
            stop=False,
        )
    for dw in range(3):
        # dh=2: even rows, row i+1
        nc.tensor.matmul(
            psum3[0:K, :, :],
            w2t3[0:64, dw, :],
            src3[0:64, 1 : 1 + ROWS_PER_CORE, dw : dw + 128 : 2],
            start=False,
            stop=(stop and dw == 2),
        )


def _build_program():
    key = (MM_DTYPE, STRATEGY)
    if key in _cache:
        return _cache[key]

    mm_dt = F32R if MM_DTYPE == "f32r" else F32

    nc = bacc.Bacc(None)
    xe = nc.declare_dram_parameter("xe", [B, C, ER, WCOLS], F32, isOutput=False)
    xo = nc.declare_dram_parameter("xo", [B, C, OR, WCOLS], F32, isOutput=False)
    wp = nc.declare_dram_parameter("wp", [128, 3, K], F32, isOutput=False)
    w2 = nc.declare_dram_parameter("w2", [64, 3, K], F32, isOutput=False)
    out = nc.declare_dram_parameter("out", [K, NOUT], F32, isOutput=True)

    with tile.TileContext(nc) as tc:
        with (
            tc.tile_pool(name="sbuf", bufs=1) as pool,
            tc.tile_pool(name="psum", bufs=1, space="PSUM") as ppool,
        ):
            wpt = pool.tile([128, 3 * K], mm_dt, tag="wpt")
            w2t = pool.tile([64, 3 * K], mm_dt, tag="w2t")
            wdma = nc.gpsimd if mm_dt is F32R else nc.sync  # SWDGE casts f32->f32r
            wdma.dma_start(out=wpt[:, :], in_=wp[:, :, :].rearrange("p a b -> p (a b)"))
            wdma.dma_start(out=w2t[:, :], in_=w2[:, :, :].rearrange("p a b -> p (a b)"))
            wpt3 = wpt[:, :].rearrange("p (a b) -> p a b", a=3)
            w2t3 = w2t[:, :].rearrange("p (a b) -> p a b", a=3)

            # staging dtype: fp32 normally; f32r (SWDGE cast DMA) for "batch"
            # strategy where matmuls consume staging directly.
            st_direct = STRATEGY == "batch" and mm_dt is F32R
            staging = pool.tile([128, B * PITCH], mm_dt if st_direct else F32,
                                tag="staging")
            st3 = staging[:, :].rearrange("p (b x) -> p b x", b=B)
            xer = xe[:, :, :, :].rearrange("b c r w -> c b (r w)")
            xor_ = xo[:, :, :, :].rearrange("b c r w -> c b (r w)")
            idma_e = nc.gpsimd if st_direct else nc.sync
            idma_o = nc.gpsimd if st_direct else nc.scalar
            for bp in range(4):
                b0 = 2 * bp
                idma_e.dma_start(
                    out=st3[0:64, b0 : b0 + 2, :], in_=xer[:, b0 : b0 + 2, :]
                )
                idma_o.dma_start(
                    out=st3[64:128, b0 : b0 + 2, 0 : OR * WCOLS],
                    in_=xor_[:, b0 : b0 + 2, :],
                )

            psum = ppool.tile([K, NOUT], F32, tag="psum")
            psum3 = psum[:, :].rearrange("p (r w) -> p r w", r=ROWS_PER_CORE)

            if STRATEGY == "batch":
                for b in range(B):
                    src3 = st3[:, b, :].rearrange("p (r w) -> p r w", r=ER)
                    _emit_conv_matmuls(
                        nc, wpt3, w2t3, src3, psum3, start=(b == 0), stop=(b == B - 1)
                    )
            elif STRATEGY == "pair":
                spair = pool.tile([128, 4 * PITCH], mm_dt, tag="spair")
                sp3 = spair[:, :].rearrange("p (b x) -> p b x", b=4)
                for bp in range(4):
                    nc.vector.tensor_add(
                        sp3[:, bp, :], st3[:, 2 * bp, :], st3[:, 2 * bp + 1, :]
                    )
                    src3 = sp3[:, bp, :].rearrange("p (r w) -> p r w", r=ER)
                    _emit_conv_matmuls(
                        nc, wpt3, w2t3, src3, psum3, start=(bp == 0), stop=(bp == 3)
                    )
            else:  # tree
                spair = pool.tile([128, 4 * PITCH], F32, tag="spair")
                sp3 = spair[:, :].rearrange("p (b x) -> p b x", b=4)
                t01 = pool.tile([128, PITCH], F32, tag="t01")
                t23 = pool.tile([128, PITCH], F32, tag="t23")
                xsum = pool.tile([128, PITCH], mm_dt, tag="xsum")
                for bp in range(2):
                    nc.vector.tensor_add(
                        sp3[:, bp, :], st3[:, 2 * bp, :], st3[:, 2 * bp + 1, :]
                    )
                nc.vector.tensor_add(t01[:, :], sp3[:, 0, :], sp3[:, 1, :])
                for bp in range(2, 4):
                    nc.vector.tensor_add(
                        sp3[:, bp, :], st3[:, 2 * bp, :], st3[:, 2 * bp + 1, :]
                    )
                nc.vector.tensor_add(t23[:, :], sp3[:, 2, :], sp3[:, 3, :])
                nc.vector.tensor_add(xsum[:, :], t01[:, :], t23[:, :])
                src3 = xsum[:, :].rearrange("p (r w) -> p r w", r=ER)
                _emit_conv_matmuls(nc, wpt3, w2t3, src3, psum3, start=True, stop=True)

            outs = pool.tile([K, NOUT], F32, tag="outs")
            nc.scalar.copy(outs[:, :], psum[:, :])
            nc.sync.dma_start(out=out[:, :], in_=outs[:, :])

    nc.compile()
    _cache[key] = nc
    return nc


def _prep_inputs(input, weight):
    inp = np.ascontiguousarray(input, dtype=np.float32)
    w = np.ascontiguousarray(weight, dtype=np.float32)

    # Padded top-left region: P[r, w] = padded coord (orig r-1, w-1)
    P = np.zeros((B, C, 130, WCOLS), np.float32)
    P[:, :, 1:129, 1:129] = inp[:, :, :128, :128]

    t = [np.ascontiguousarray(w[:, :, dh, :].transpose(1, 2, 0)) for dh in range(3)]
    wp_host = np.ascontiguousarray(np.concatenate([t[0], t[1]], axis=0))
    w2_host = t[2]

    in_maps = []
    for q in range(NCORES):
        r0 = 16 * q
        xe_q = np.ascontiguousarray(P[:, :, r0 : r0 + 17 : 2, :])
        xo_q = np.ascontiguousarray(P[:, :, r0 + 1 : r0 + 16 : 2, :])
        in_maps.append({"xe": xe_q, "xo": xo_q, "wp": wp_host, "w2": w2_host})
    return in_maps


def kernel(input, weight):
    global LAST_EXEC_NS
    nc = _build_program()
    in_maps = _prep_inputs(input, weight)
    res = run_bass_kernel_spmd(nc, in_maps, list(range(NCORES)), trace=TRACE)
    LAST_EXEC_NS = res.exec_time_ns

    vals = np.concatenate(
        [res.results[q]["out"].reshape(K, ROWS_PER_CORE, 64) for q in range(NCORES)],
        axis=1,
    )  # (K, 64, 64)
    out = np.zeros((B, K, 128, 128), np.float32)
    out[:, :, ::2, ::2] = vals[None]
    return out


# revision 3
# speedup vs baseline: 1.2397x; 1.2111x over previous
"""Trainium2 Bass kernel for nn_Conv2d_71476845740806.

Reference semantics (buggy naive Conv2d):
  xsum = pad(input, 1).sum(batch)                  # (1, C, 258, 258)
  conv = conv2d(xsum, weight, stride=2, VALID)     # (1, K, 128, 128)
  vals = conv[0, :, :64, :64]                      # (K, 64, 64)
  out  = zeros(B, K, 128, 128); out[:, :, ::2, ::2] = vals  (batch-replicated)

Only window starts (2i, 2j), i,j in [0,64) are used -> only padded rows/cols
0..128 of the summed image matter -> only input rows/cols 0..127.

Device strategy (8 cores, SPMD):
  - Shard the 64 output rows: core q computes rows 8q..8q+7 for ALL K=128
    filters. Needs padded rows 16q..16q+16 (17 rows) x 129 cols, all b, c.
  - Host preps one combined per-core tensor xc[128, 8, 1170]:
    partitions 0..63 = (c, even padded rows 0..8 x 130 cols),
    partitions 64..127 = (c, odd padded rows 0..7 x 130 cols, zero-padded).
    Contiguous per (partition, batch) -> near-peak DMA efficiency.
  - Batch-sum via DVE tensor_add (casting to the matmul dtype on write).
  - Conv as 6 matmuls per accumulation group into one PSUM bank [128, 512]:
      3 x contract-128 (kernel rows dh=0,1 paired across partition halves)
      3 x contract-64  (dh=2, even partitions only, shifted one row)
    rhs AP does the stride-2 column access directly: [.., (8 rows), (64 cols step 2)].
  - DMA out per-core vals (128, 8*64); host scatters into the zero output.
"""

import ml_dtypes
import numpy as np

import concourse.bacc as bacc
import concourse.bass as bass
import concourse.mybir as mybir
from concourse import tile
from concourse.bass_utils import run_bass_kernel_spmd

F32 = mybir.dt.float32
F32R = mybir.dt.float32r
BF16 = mybir.dt.bfloat16

B, C, H, W = 8, 64, 256, 256
K = 128
NCORES = 8
ROWS_PER_CORE = 8          # output rows per core (64 total)
ER = 9                     # even padded rows per core
OR = 8                     # odd padded rows per core
WCOLS = 130                # stored padded cols 0..129 (used: 0..128)
PITCH = ER * WCOLS         # 1170 per-batch free pitch
NOUT = ROWS_PER_CORE * 64  # 512

# Matmul input dtype: "fp32" (exact, 4 cyc/row), "f32r" (1 cyc/row),
# "bf16" (1 cyc/row, HAM-warmable).
MM_DTYPE = "f32r"
# Sum strategy: "tree" = full sum then 6 matmuls; "quad" = 2 groups of 4
# batches (12 matmuls); "pair" = 4 groups of 2 batches (24 matmuls).
STRATEGY = "pair"
# Input DMA chunks (must divide 8): 4 = batch pairs, 8 = single batches.
NCHUNK = 4

TRACE = False
LAST_EXEC_NS = None

_cache = {}


def _mm_np_dtype():
    return ml_dtypes.bfloat16 if MM_DTYPE == "bf16" else np.float32


def _emit_conv_matmuls(nc, wpt3, w2t3, src3, psum3, start, stop):
    """6 matmuls of the 3x3 stride-2 conv of src into psum (accumulating).

    src3: [128, ER, WCOLS] AP; partitions 0..63 = (c, even rows),
          64..127 = (c, odd rows; row index i is padded row 2i+1).
    """
    for dw in range(3):
        # dh=0 (even rows, row i) paired with dh=1 (odd rows, row i)
        nc.tensor.matmul(
            psum3[:, :, :],
            wpt3[:, dw, :],
            src3[:, 0:ROWS_PER_CORE, dw : dw + 128 : 2],
            start=(start and dw == 0),
            stop=False,
        )
    for dw in range(3):
        # dh=2: even rows, row i+1
        nc.tensor.matmul(
            psum3[0:K, :, :],
            w2t3[0:64, dw, :],
            src3[0:64, 1 : 1 + ROWS_PER_CORE, dw : dw + 128 : 2],
            start=False,
            stop=(stop and dw == 2),
        )


def _build_program():
    key = (MM_DTYPE, STRATEGY, NCHUNK)
    if key in _cache:
        return _cache[key]

    mm_dt = {"fp32": F32, "f32r": F32R, "bf16": BF16}[MM_DTYPE]
    w_dram_dt = BF16 if mm_dt is BF16 else F32

    nc = bacc.Bacc(None)
    xc = nc.declare_dram_parameter("xc", [128, B, PITCH], F32, isOutput=False)
    wp = nc.declare_dram_parameter("wp", [128, 3, K], w_dram_dt, isOutput=False)
    w2 = nc.declare_dram_parameter("w2", [64, 3, K], w_dram_dt, isOutput=False)
    out = nc.declare_dram_parameter("out", [K, NOUT], F32, isOutput=True)

    bpc = B // NCHUNK  # batches per DMA chunk

    with tile.TileContext(nc) as tc:
        with (
            tc.tile_pool(name="sbuf", bufs=1) as pool,
            tc.tile_pool(name="psum", bufs=1, space="PSUM") as ppool,
        ):
            wpt = pool.tile([128, 3 * K], mm_dt, tag="wpt")
            w2t = pool.tile([64, 3 * K], mm_dt, tag="w2t")
            # f32r needs a rounding producer -> SWDGE cast DMA; bf16/fp32 are
            # plain HWDGE copies.
            wdma = nc.gpsimd if mm_dt is F32R else nc.sync
            wdma.dma_start(out=wpt[:, :], in_=wp[:, :, :].rearrange("p a b -> p (a b)"))
            wdma.dma_start(out=w2t[:, :], in_=w2[:, :, :].rearrange("p a b -> p (a b)"))
            wpt3 = wpt[:, :].rearrange("p (a b) -> p a b", a=3)
            w2t3 = w2t[:, :].rearrange("p (a b) -> p a b", a=3)

            staging = pool.tile([128, B * PITCH], F32, tag="staging")
            st3 = staging[:, :].rearrange("p (b x) -> p b x", b=B)
            xcr = xc[:, :, :]
            for ch in range(NCHUNK):
                b0 = ch * bpc
                eng = nc.sync if ch % 2 == 0 else nc.scalar
                eng.dma_start(
                    out=st3[:, b0 : b0 + bpc, :], in_=xcr[:, b0 : b0 + bpc, :]
                )

            psum = ppool.tile([K, NOUT], F32, tag="psum")
            psum3 = psum[:, :].rearrange("p (r w) -> p r w", r=ROWS_PER_CORE)

            spair = pool.tile([128, 4 * PITCH], mm_dt if STRATEGY == "pair" else F32,
                              tag="spair")
            sp3 = spair[:, :].rearrange("p (b x) -> p b x", b=4)

            if STRATEGY == "pair":
                for bp in range(4):
                    nc.vector.tensor_add(
                        sp3[:, bp, :], st3[:, 2 * bp, :], st3[:, 2 * bp + 1, :]
                    )
                    src3 = sp3[:, bp, :].rearrange("p (r w) -> p r w", r=ER)
                    _emit_conv_matmuls(
                        nc, wpt3, w2t3, src3, psum3, start=(bp == 0), stop=(bp == 3)
                    )
            elif STRATEGY == "quad":
                quad = pool.tile([128, 2 * PITCH], mm_dt, tag="quad")
                q3 = quad[:, :].rearrange("p (b x) -> p b x", b=2)
                for h in range(2):
                    for bp in (2 * h, 2 * h + 1):
                        nc.vector.tensor_add(
                            sp3[:, bp, :], st3[:, 2 * bp, :], st3[:, 2 * bp + 1, :]
                        )
                    nc.vector.tensor_add(
                        q3[:, h, :], sp3[:, 2 * h, :], sp3[:, 2 * h + 1, :]
                    )
                    src3 = q3[:, h, :].rearrange("p (r w) -> p r w", r=ER)
                    _emit_conv_matmuls(
                        nc, wpt3, w2t3, src3, psum3, start=(h == 0), stop=(h == 1)
                    )
            else:  # tree
                t01 = pool.tile([128, PITCH], F32, tag="t01")
                t23 = pool.tile([128, PITCH], F32, tag="t23")
                xsum = pool.tile([128, PITCH], mm_dt, tag="xsum")
                for bp in range(2):
                    nc.vector.tensor_add(
                        sp3[:, bp, :], st3[:, 2 * bp, :], st3[:, 2 * bp + 1, :]
                    )
                nc.vector.tensor_add(t01[:, :], sp3[:, 0, :], sp3[:, 1, :])
                for bp in range(2, 4):
                    nc.vector.tensor_add(
                        sp3[:, bp, :], st3[:, 2 * bp, :], st3[:, 2 * bp + 1, :]
                    )
                nc.vector.tensor_add(t23[:, :], sp3[:, 2, :], sp3[:, 3, :])
                nc.vector.tensor_add(xsum[:, :], t01[:, :], t23[:, :])
                src3 = xsum[:, :].rearrange("p (r w) -> p r w", r=ER)
                _emit_conv_matmuls(nc, wpt3, w2t3, src3, psum3, start=True, stop=True)

            outs = pool.tile([K, NOUT], F32, tag="outs")
            nc.vector.tensor_copy(outs[:, :], psum[:, :])
            nc.sync.dma_start(out=out[:, :], in_=outs[:, :])

    nc.compile()
    _cache[key] = nc
    return nc


def _prep_inputs(input, weight):
    inp = np.ascontiguousarray(input, dtype=np.float32)
    w = np.ascontiguousarray(weight, dtype=np.float32)

    # Padded top-left region: P[r, w] = padded coord (orig r-1, w-1)
    P = np.zeros((B, C, 130, WCOLS), np.float32)
    P[:, :, 1:129, 1:129] = inp[:, :, :128, :128]
    Pc = np.ascontiguousarray(P.transpose(1, 0, 2, 3))  # (C, B, 130, WCOLS)

    wdt = _mm_np_dtype()
    t = [np.ascontiguousarray(w[:, :, dh, :].transpose(1, 2, 0)).astype(wdt)
         for dh in range(3)]
    wp_host = np.ascontiguousarray(np.concatenate([t[0], t[1]], axis=0))
    w2_host = t[2]

    in_maps = []
    for q in range(NCORES):
        r0 = 16 * q
        xcq = np.zeros((128, B, PITCH), np.float32)
        xcq[0:64] = Pc[:, :, r0 : r0 + 17 : 2, :].reshape(64, B, PITCH)
        xcq[64:128, :, 0 : OR * WCOLS] = Pc[:, :, r0 + 1 : r0 + 16 : 2, :].reshape(
            64, B, OR * WCOLS
        )
        in_maps.append({"xc": xcq, "wp": wp_host, "w2": w2_host})
    return in_maps


def kernel(input, weight):
    global LAST_EXEC_NS
    nc = _build_program()
    in_maps = _prep_inputs(input, weight)
    res = run_bass_kernel_spmd(nc, in_maps, list(range(NCORES)), trace=TRACE)
    LAST_EXEC_NS = res.exec_time_ns

    vals = np.concatenate(
        [res.results[q]["out"].reshape(K, ROWS_PER_CORE, 64) for q in range(NCORES)],
        axis=1,
    )  # (K, 64, 64)
    out = np.zeros((B, K, 128, 128), np.float32)
    out[:, :, ::2, ::2] = vals[None]
    return out


# revision 4
# speedup vs baseline: 1.4525x; 1.1717x over previous
"""Trainium2 Bass kernel for nn_Conv2d_71476845740806.

Reference semantics (buggy naive Conv2d):
  xsum = pad(input, 1).sum(batch)                  # (1, C, 258, 258)
  conv = conv2d(xsum, weight, stride=2, VALID)     # (1, K, 128, 128)
  vals = conv[0, :, :64, :64]                      # (K, 64, 64)
  out  = zeros(B, K, 128, 128); out[:, :, ::2, ::2] = vals  (batch-replicated)

Only window starts (2i, 2j), i,j in [0,64) are used -> only padded rows/cols
0..128 of the summed image matter -> only input rows/cols 0..127.

Device strategy (8 cores, SPMD):
  - Shard the 64 output rows: core q computes rows 8q..8q+7 for ALL K=128
    filters. Needs padded rows 16q..16q+16 (17 rows) x 129 cols, all b, c.
  - Host preps one combined per-core tensor xc[128, 8, 1170]:
    partitions 0..63 = (c, even padded rows 0..8 x 130 cols),
    partitions 64..127 = (c, odd padded rows 0..7 x 130 cols, zero-padded).
    Contiguous per (partition, batch) -> near-peak DMA efficiency.
  - Batch-sum via DVE tensor_add (casting to the matmul dtype on write).
  - Conv as 6 matmuls per accumulation group into one PSUM bank [128, 512]:
      3 x contract-128 (kernel rows dh=0,1 paired across partition halves)
      3 x contract-64  (dh=2, even partitions only, shifted one row)
    rhs AP does the stride-2 column access directly: [.., (8 rows), (64 cols step 2)].
  - DMA out per-core vals (128, 8*64); host scatters into the zero output.
"""

import ml_dtypes
import numpy as np

import concourse.bacc as bacc
import concourse.bass as bass
import concourse.mybir as mybir
from concourse import tile
from concourse.bass_utils import run_bass_kernel_spmd

F32 = mybir.dt.float32
F32R = mybir.dt.float32r
BF16 = mybir.dt.bfloat16

B, C, H, W = 8, 64, 256, 256
K = 128
NCORES = 8
ROWS_PER_CORE = 8          # output rows per core (64 total)
ER = 9                     # even padded rows per core
OR = 8                     # odd padded rows per core
WCOLS = 130                # stored padded cols 0..129 (used: 0..128)
PITCH = ER * WCOLS         # 1170 per-batch free pitch
NOUT = ROWS_PER_CORE * 64  # 512

# Matmul input dtype: "fp32" (exact, 4 cyc/row), "f32r" (1 cyc/row),
# "bf16" (1 cyc/row, HAM-warmable).
MM_DTYPE = "f32r"
# Sum strategy: "tree" = full sum then 6 matmuls; "quad" = 2 groups of 4
# batches (12 matmuls); "pair" = 4 groups of 2 batches (24 matmuls).
STRATEGY = "pair"
# Input DMA chunks (must divide 8): 4 = batch pairs, 8 = single batches.
NCHUNK = 4

TRACE = False
LAST_EXEC_NS = None

_cache = {}


def _mm_np_dtype():
    return ml_dtypes.bfloat16 if MM_DTYPE == "bf16" else np.float32


def _emit_conv_matmuls(nc, wpt3, w2t3, src3, psum3, start, stop):
    """6 matmuls of the 3x3 stride-2 conv of src into psum (accumulating).

    src3: [128, ER, WCOLS] AP; partitions 0..63 = (c, even rows),
          64..127 = (c, odd rows; row index i is padded row 2i+1).
    """
    for dw in range(3):
        # dh=0 (even rows, row i) paired with dh=1 (odd rows, row i)
        nc.tensor.matmul(
            psum3[:, :, :],
            wpt3[:, dw, :],
            src3[:, 0:ROWS_PER_CORE, dw : dw + 128 : 2],
            start=(start and dw == 0),
            stop=False,
        )
    for dw in range(3):
        # dh=2: even rows, row i+1
        nc.tensor.matmul(
            psum3[0:K, :, :],
            w2t3[0:64, dw, :],
            src3[0:64, 1 : 1 + ROWS_PER_CORE, dw : dw + 128 : 2],
            start=False,
            stop=(stop and dw == 2),
        )


def _build_program():
    key = (MM_DTYPE, STRATEGY, NCHUNK)
    if key in _cache:
        return _cache[key]

    mm_dt = {"fp32": F32, "f32r": F32R, "bf16": BF16}[MM_DTYPE]
    w_dram_dt = BF16 if mm_dt is BF16 else F32

    nc = bacc.Bacc(None)
    xc = nc.declare_dram_parameter("xc", [128, B, PITCH], F32, isOutput=False)
    wp = nc.declare_dram_parameter("wp", [128, 3, K], w_dram_dt, isOutput=False)
    w2 = nc.declare_dram_parameter("w2", [64, 3, K], w_dram_dt, isOutput=False)
    out = nc.declare_dram_parameter("out", [K, NOUT], F32, isOutput=True)

    bpc = B // NCHUNK  # batches per DMA chunk

    with tile.TileContext(nc) as tc:
        with (
            tc.tile_pool(name="sbuf", bufs=1) as pool,
            tc.tile_pool(name="psum", bufs=1, space="PSUM") as ppool,
        ):
            wpt = pool.tile([128, 3 * K], mm_dt, tag="wpt")
            w2t = pool.tile([64, 3 * K], mm_dt, tag="w2t")
            # f32r needs a rounding producer -> SWDGE cast DMA; bf16/fp32 are
            # plain HWDGE copies.
            wdma = nc.gpsimd if mm_dt is F32R else nc.sync
            wdma.dma_start(out=wpt[:, :], in_=wp[:, :, :].rearrange("p a b -> p (a b)"))
            wdma.dma_start(out=w2t[:, :], in_=w2[:, :, :].rearrange("p a b -> p (a b)"))
            wpt3 = wpt[:, :].rearrange("p (a b) -> p a b", a=3)
            w2t3 = w2t[:, :].rearrange("p (a b) -> p a b", a=3)

            staging = pool.tile([128, B * PITCH], F32, tag="staging")
            st3 = staging[:, :].rearrange("p (b x) -> p b x", b=B)
            xcr = xc[:, :, :]
            # All input chunks on ONE HWDGE queue: strict FIFO makes chunk i
            # complete before chunk i+1 starts, so the adds pipeline. (Two
            # queues interleave at packet granularity and everything lands
            # at the same late time.)
            for ch in range(NCHUNK):
                b0 = ch * bpc
                nc.sync.dma_start(
                    out=st3[:, b0 : b0 + bpc, :], in_=xcr[:, b0 : b0 + bpc, :]
                )

            psum = ppool.tile([K, NOUT], F32, tag="psum")
            psum3 = psum[:, :].rearrange("p (r w) -> p r w", r=ROWS_PER_CORE)

            spair = pool.tile([128, 4 * PITCH], mm_dt if STRATEGY == "pair" else F32,
                              tag="spair")
            sp3 = spair[:, :].rearrange("p (b x) -> p b x", b=4)

            if STRATEGY == "pair":
                for bp in range(4):
                    nc.vector.tensor_add(
                        sp3[:, bp, :], st3[:, 2 * bp, :], st3[:, 2 * bp + 1, :]
                    )
                    src3 = sp3[:, bp, :].rearrange("p (r w) -> p r w", r=ER)
                    _emit_conv_matmuls(
                        nc, wpt3, w2t3, src3, psum3, start=(bp == 0), stop=(bp == 3)
                    )
            elif STRATEGY == "quad":
                quad = pool.tile([128, 2 * PITCH], mm_dt, tag="quad")
                q3 = quad[:, :].rearrange("p (b x) -> p b x", b=2)
                for h in range(2):
                    for bp in (2 * h, 2 * h + 1):
                        nc.vector.tensor_add(
                            sp3[:, bp, :], st3[:, 2 * bp, :], st3[:, 2 * bp + 1, :]
                        )
                    nc.vector.tensor_add(
                        q3[:, h, :], sp3[:, 2 * h, :], sp3[:, 2 * h + 1, :]
                    )
                    src3 = q3[:, h, :].rearrange("p (r w) -> p r w", r=ER)
                    _emit_conv_matmuls(
                        nc, wpt3, w2t3, src3, psum3, start=(h == 0), stop=(h == 1)
                    )
            else:  # tree
                t01 = pool.tile([128, PITCH], F32, tag="t01")
                t23 = pool.tile([128, PITCH], F32, tag="t23")
                xsum = pool.tile([128, PITCH], mm_dt, tag="xsum")
                for bp in range(2):
                    nc.vector.tensor_add(
                        sp3[:, bp, :], st3[:, 2 * bp, :], st3[:, 2 * bp + 1, :]
                    )
                nc.vector.tensor_add(t01[:, :], sp3[:, 0, :], sp3[:, 1, :])
                for bp in range(2, 4):
                    nc.vector.tensor_add(
                        sp3[:, bp, :], st3[:, 2 * bp, :], st3[:, 2 * bp + 1, :]
                    )
                nc.vector.tensor_add(t23[:, :], sp3[:, 2, :], sp3[:, 3, :])
                nc.vector.tensor_add(xsum[:, :], t01[:, :], t23[:, :])
                src3 = xsum[:, :].rearrange("p (r w) -> p r w", r=ER)
                _emit_conv_matmuls(nc, wpt3, w2t3, src3, psum3, start=True, stop=True)

            outs = pool.tile([K, NOUT], F32, tag="outs")
            nc.vector.tensor_copy(outs[:, :], psum[:, :])
            nc.sync.dma_start(out=out[:, :], in_=outs[:, :])

    nc.compile()
    _cache[key] = nc
    return nc


def _prep_inputs(input, weight):
    inp = np.ascontiguousarray(input, dtype=np.float32)
    w = np.ascontiguousarray(weight, dtype=np.float32)

    # Padded top-left region: P[r, w] = padded coord (orig r-1, w-1)
    P = np.zeros((B, C, 130, WCOLS), np.float32)
    P[:, :, 1:129, 1:129] = inp[:, :, :128, :128]
    Pc = np.ascontiguousarray(P.transpose(1, 0, 2, 3))  # (C, B, 130, WCOLS)

    wdt = _mm_np_dtype()
    t = [np.ascontiguousarray(w[:, :, dh, :].transpose(1, 2, 0)).astype(wdt)
         for dh in range(3)]
    wp_host = np.ascontiguousarray(np.concatenate([t[0], t[1]], axis=0))
    w2_host = t[2]

    in_maps = []
    for q in range(NCORES):
        r0 = 16 * q
        xcq = np.zeros((128, B, PITCH), np.float32)
        xcq[0:64] = Pc[:, :, r0 : r0 + 17 : 2, :].reshape(64, B, PITCH)
        xcq[64:128, :, 0 : OR * WCOLS] = Pc[:, :, r0 + 1 : r0 + 16 : 2, :].reshape(
            64, B, OR * WCOLS
        )
        in_maps.append({"xc": xcq, "wp": wp_host, "w2": w2_host})
    return in_maps


def kernel(input, weight):
    global LAST_EXEC_NS
    nc = _build_program()
    in_maps = _prep_inputs(input, weight)
    res = run_bass_kernel_spmd(nc, in_maps, list(range(NCORES)), trace=TRACE)
    LAST_EXEC_NS = res.exec_time_ns

    vals = np.concatenate(
        [res.results[q]["out"].reshape(K, ROWS_PER_CORE, 64) for q in range(NCORES)],
        axis=1,
    )  # (K, 64, 64)
    out = np.zeros((B, K, 128, 128), np.float32)
    out[:, :, ::2, ::2] = vals[None]
    return out


# revision 7
# speedup vs baseline: 1.4698x; 1.0119x over previous
"""Trainium2 Bass kernel for nn_Conv2d_71476845740806.

Reference semantics (buggy naive Conv2d):
  xsum = pad(input, 1).sum(batch)                  # (1, C, 258, 258)
  conv = conv2d(xsum, weight, stride=2, VALID)     # (1, K, 128, 128)
  vals = conv[0, :, :64, :64]                      # (K, 64, 64)
  out  = zeros(B, K, 128, 128); out[:, :, ::2, ::2] = vals  (batch-replicated)

Only window starts (2i, 2j), i,j in [0,64) are used -> only padded rows/cols
0..128 of the summed image matter -> only input rows/cols 0..127.

Device strategy (8 cores, SPMD):
  - Shard the 64 output rows: core q computes rows 8q..8q+7 for ALL K=128
    filters. Needs padded rows 16q..16q+16 (17 rows) x 129 cols, all b, c.
  - Host preps one combined per-core tensor xc[128, 8, 1170]:
    partitions 0..63 = (c, even padded rows 0..8 x 130 cols),
    partitions 64..127 = (c, odd padded rows 0..7 x 130 cols, zero-padded).
    Contiguous per (partition, batch) -> near-peak DMA efficiency.
  - Batch-sum via DVE tensor_add (casting to the matmul dtype on write).
  - Conv as 6 matmuls per accumulation group into one PSUM bank [128, 512]:
      3 x contract-128 (kernel rows dh=0,1 paired across partition halves)
      3 x contract-64  (dh=2, even partitions only, shifted one row)
    rhs AP does the stride-2 column access directly: [.., (8 rows), (64 cols step 2)].
  - DMA out per-core vals (128, 8*64); host scatters into the zero output.
"""

import ml_dtypes
import numpy as np

import concourse.bacc as bacc
import concourse.bass as bass
import concourse.mybir as mybir
from concourse import tile
from concourse.bass_utils import run_bass_kernel_spmd

F32 = mybir.dt.float32
F32R = mybir.dt.float32r
BF16 = mybir.dt.bfloat16

B, C, H, W = 8, 64, 256, 256
K = 128
NCORES = 8
ROWS_PER_CORE = 8          # output rows per core (64 total)
ER = 9                     # even padded rows per core
OR = 8                     # odd padded rows per core
WCOLS = 130                # stored padded cols 0..129 (used: 0..128)
PITCH = ER * WCOLS         # 1170 per-batch free pitch
NOUT = ROWS_PER_CORE * 64  # 512

# Matmul input dtype: "fp32" (exact, 4 cyc/row), "f32r" (1 cyc/row),
# "bf16" (1 cyc/row, HAM-warmable).
MM_DTYPE = "f32r"
# Sum strategy: "tree" = full sum then 6 matmuls; "quad" = 2 groups of 4
# batches (12 matmuls); "pair" = 4 groups of 2 batches (24 matmuls).
STRATEGY = "pair"
# Input DMA chunks (must divide 8): 4 = batch pairs, 8 = single batches.
NCHUNK = 8
# Dummy bf16 matmuls (on already-landed chunk-0 data) to lift the PE HAM
# clock gate to 8/8 before the real matmuls start. 0 = off.
WARMUP = 0

TRACE = False
LAST_EXEC_NS = None

_cache = {}


def _mm_np_dtype():
    return ml_dtypes.bfloat16 if MM_DTYPE == "bf16" else np.float32


def _emit_conv_matmuls(nc, wpt3, w2t3, src3, psum3, start, stop):
    """6 matmuls of the 3x3 stride-2 conv of src into psum (accumulating).

    src3: [128, ER, WCOLS] AP; partitions 0..63 = (c, even rows),
          64..127 = (c, odd rows; row index i is padded row 2i+1).
    """
    for dw in range(3):
        # dh=0 (even rows, row i) paired with dh=1 (odd rows, row i)
        nc.tensor.matmul(
            psum3[:, :, :],
            wpt3[:, dw, :],
            src3[:, 0:ROWS_PER_CORE, dw : dw + 128 : 2],
            start=(start and dw == 0),
            stop=False,
        )
    for dw in range(3):
        # dh=2: even rows, row i+1
        nc.tensor.matmul(
            psum3[0:K, :, :],
            w2t3[0:64, dw, :],
            src3[0:64, 1 : 1 + ROWS_PER_CORE, dw : dw + 128 : 2],
            start=False,
            stop=(stop and dw == 2),
        )


def _build_program():
    key = (MM_DTYPE, STRATEGY, NCHUNK, WARMUP)
    if key in _cache:
        return _cache[key]

    mm_dt = {"fp32": F32, "f32r": F32R, "bf16": BF16}[MM_DTYPE]
    w_dram_dt = BF16 if mm_dt is BF16 else F32

    nc = bacc.Bacc(None)
    xc = nc.declare_dram_parameter("xc", [128, B, PITCH], F32, isOutput=False)
    wp = nc.declare_dram_parameter("wp", [128, 3, K], w_dram_dt, isOutput=False)
    w2 = nc.declare_dram_parameter("w2", [64, 3, K], w_dram_dt, isOutput=False)
    out = nc.declare_dram_parameter("out", [K, NOUT], F32, isOutput=True)

    bpc = B // NCHUNK  # batches per DMA chunk

    with tile.TileContext(nc) as tc:
        with (
            tc.tile_pool(name="sbuf", bufs=1) as pool,
            tc.tile_pool(name="psum", bufs=1, space="PSUM") as ppool,
        ):
            wpt = pool.tile([128, 3 * K], mm_dt, tag="wpt")
            w2t = pool.tile([64, 3 * K], mm_dt, tag="w2t")
            # f32r needs a rounding producer -> SWDGE cast DMA; bf16/fp32 are
            # plain HWDGE copies.
            wdma = nc.gpsimd if mm_dt is F32R else nc.sync
            wdma.dma_start(out=wpt[:, :], in_=wp[:, :, :].rearrange("p a b -> p (a b)"))
            wdma.dma_start(out=w2t[:, :], in_=w2[:, :, :].rearrange("p a b -> p (a b)"))
            wpt3 = wpt[:, :].rearrange("p (a b) -> p a b", a=3)
            w2t3 = w2t[:, :].rearrange("p (a b) -> p a b", a=3)

            staging = pool.tile([128, B * PITCH], F32, tag="staging")
            st3 = staging[:, :].rearrange("p (b x) -> p b x", b=B)
            xcr = xc[:, :, :]
            # All input chunks on ONE HWDGE queue: strict FIFO makes chunk i
            # complete before chunk i+1 starts, so the adds pipeline. (Two
            # queues interleave at packet granularity and everything lands
            # at the same late time.)
            for ch in range(NCHUNK):
                b0 = ch * bpc
                nc.sync.dma_start(
                    out=st3[:, b0 : b0 + bpc, :], in_=xcr[:, b0 : b0 + bpc, :]
                )

            psum = ppool.tile([K, NOUT], F32, tag="psum")
            psum3 = psum[:, :].rearrange("p (r w) -> p r w", r=ROWS_PER_CORE)

            if WARMUP:
                # Reads chunk-0 staging data reinterpreted as bf16 -> starts
                # as soon as the first chunk lands, keeps the PE busy (bf16
                # counts for HAM) until the real matmuls begin.
                dpsum = ppool.tile([K, NOUT], F32, tag="dpsum")
                wu_lhs = staging[:, 0:64].bitcast(BF16)
                wu_rhs = staging[:, 64:320].bitcast(BF16)
                for i in range(WARMUP):
                    nc.tensor.matmul(
                        dpsum[:, :], wu_lhs, wu_rhs,
                        start=(i == 0), stop=(i == WARMUP - 1),
                    )

            spair = pool.tile([128, 4 * PITCH], mm_dt if STRATEGY == "pair" else F32,
                              tag="spair")
            sp3 = spair[:, :].rearrange("p (b x) -> p b x", b=4)

            if STRATEGY == "pair":
                for bp in range(4):
                    nc.vector.tensor_add(
                        sp3[:, bp, :], st3[:, 2 * bp, :], st3[:, 2 * bp + 1, :]
                    )
                    src3 = sp3[:, bp, :].rearrange("p (r w) -> p r w", r=ER)
                    _emit_conv_matmuls(
                        nc, wpt3, w2t3, src3, psum3, start=(bp == 0), stop=(bp == 3)
                    )
            elif STRATEGY == "quad":
                quad = pool.tile([128, 2 * PITCH], mm_dt, tag="quad")
                q3 = quad[:, :].rearrange("p (b x) -> p b x", b=2)
                for h in range(2):
                    for bp in (2 * h, 2 * h + 1):
                        nc.vector.tensor_add(
                            sp3[:, bp, :], st3[:, 2 * bp, :], st3[:, 2 * bp + 1, :]
                        )
                    nc.vector.tensor_add(
                        q3[:, h, :], sp3[:, 2 * h, :], sp3[:, 2 * h + 1, :]
                    )
                    src3 = q3[:, h, :].rearrange("p (r w) -> p r w", r=ER)
                    _emit_conv_matmuls(
                        nc, wpt3, w2t3, src3, psum3, start=(h == 0), stop=(h == 1)
                    )
            else:  # tree
                t01 = pool.tile([128, PITCH], F32, tag="t01")
                t23 = pool.tile([128, PITCH], F32, tag="t23")
                xsum = pool.tile([128, PITCH], mm_dt, tag="xsum")
                for bp in range(2):
                    nc.vector.tensor_add(
                        sp3[:, bp, :], st3[:, 2 * bp, :], st3[:, 2 * bp + 1, :]
                    )
                nc.vector.tensor_add(t01[:, :], sp3[:, 0, :], sp3[:, 1, :])
                for bp in range(2, 4):
                    nc.vector.tensor_add(
                        sp3[:, bp, :], st3[:, 2 * bp, :], st3[:, 2 * bp + 1, :]
                    )
                nc.vector.tensor_add(t23[:, :], sp3[:, 2, :], sp3[:, 3, :])
                nc.vector.tensor_add(xsum[:, :], t01[:, :], t23[:, :])
                src3 = xsum[:, :].rearrange("p (r w) -> p r w", r=ER)
                _emit_conv_matmuls(nc, wpt3, w2t3, src3, psum3, start=True, stop=True)

            outs = pool.tile([K, NOUT], F32, tag="outs")
            nc.vector.tensor_copy(outs[:, :], psum[:, :])
            nc.sync.dma_start(out=out[:, :], in_=outs[:, :])

    nc.compile()
    _cache[key] = nc
    return nc


def _prep_inputs(input, weight):
    inp = np.ascontiguousarray(input, dtype=np.float32)
    w = np.ascontiguousarray(weight, dtype=np.float32)

    # Padded top-left region: P[r, w] = padded coord (orig r-1, w-1)
    P = np.zeros((B, C, 130, WCOLS), np.float32)
    P[:, :, 1:129, 1:129] = inp[:, :, :128, :128]
    Pc = np.ascontiguousarray(P.transpose(1, 0, 2, 3))  # (C, B, 130, WCOLS)

    wdt = _mm_np_dtype()
    t = [np.ascontiguousarray(w[:, :, dh, :].transpose(1, 2, 0)).astype(wdt)
         for dh in range(3)]
    wp_host = np.ascontiguousarray(np.concatenate([t[0], t[1]], axis=0))
    w2_host = t[2]

    in_maps = []
    for q in range(NCORES):
        r0 = 16 * q
        xcq = np.zeros((128, B, PITCH), np.float32)
        xcq[0:64] = Pc[:, :, r0 : r0 + 17 : 2, :].reshape(64, B, PITCH)
        xcq[64:128, :, 0 : OR * WCOLS] = Pc[:, :, r0 + 1 : r0 + 16 : 2, :].reshape(
            64, B, OR * WCOLS
        )
        in_maps.append({"xc": xcq, "wp": wp_host, "w2": w2_host})
    return in_maps


def kernel(input, weight):
    global LAST_EXEC_NS
    nc = _build_program()
    in_maps = _prep_inputs(input, weight)
    res = run_bass_kernel_spmd(nc, in_maps, list(range(NCORES)), trace=TRACE)
    LAST_EXEC_NS = res.exec_time_ns

    vals = np.concatenate(
        [res.results[q]["out"].reshape(K, ROWS_PER_CORE, 64) for q in range(NCORES)],
        axis=1,
    )  # (K, 64, 64)
    out = np.zeros((B, K, 128, 128), np.float32)
    out[:, :, ::2, ::2] = vals[None]
    return out
